# revision 19
# baseline (speedup 1.0000x reference)
"""ASTNN-style GNN message passing kernel for 8 Trainium2 NeuronCores.

Problem shapes (hardcoded; see module constants):
  131072 nodes = 4096 trees x 32 nodes; 4096 trees = 64 programs x 64 statements.
  Pipeline: emb gather -> linear -> bottom-up subtree sums -> per-tree max ->
  bidirectional GRU over 64 steps (batch 64) -> max pool -> FC.

Sharding: data-parallel by trees/programs. Core c owns programs [8c, 8c+8) =
512 trees = 16384 nodes. No collectives; host gathers the 8 [8,104] outputs.

Key algebraic restructure: the reference's level-by-level scatter-add equals
S = A @ (G @ W) + s * b^T per tree, where A[i,j]=1 iff j is in the subtree of
i (host-built from parent indices - integer-only preprocessing), G = gathered
embedding rows, s_i = subtree size. Per 4-tree 128-node block (no transposes):
  M1^T[e,i] = sum_j G[j,e] AT[j,i]      (PE: lhsT=G block, rhs=AT blockdiag)
  S^T[c,i]  = sum_e W[e,c] M1^T[e,i]    (PE: lhsT=W_lin as-is)
  S^T[c,i] += b[c] * s[i]               (PE: K=1 accumulate)
Per-tree max = segmented reduce_max over 32-col groups; stmt cols are l-major
(col = l*8 + b) so every GRU step's batch slice is contiguous.

GRU: gates-on-partitions layout. gi (x-side) and all biases accumulate into
PSUM banks once; each step's h-matmuls accumulate into the same banks so the
sigmoid/tanh read directly from PSUM. Groups of 64 stmt cols flow from phase
A to the GRU in order 0,7,1,6,... so the forward GRU consumes from the front
while the backward GRU consumes from the back, overlapping the gather.

Matmuls use float32r (bitcast of fp32): 1 instr instead of fp32's 2, and
1 cycle/row when the moving dim is >=256.
"""

import numpy as np

import concourse.bacc as bacc
import concourse.bass as bass
import concourse.mybir as mybir
import concourse.tile as tile
from concourse.bass_utils import run_bass_kernel_spmd

# Problem constants
B, L, N = 64, 64, 32
V, E, C, H, OUT = 100000, 128, 128, 100, 104
NCORES = 8
PB = B // NCORES          # programs per core = 8
TREES_PC = PB * L         # trees per core = 512
NODES_PC = TREES_PC * N   # nodes per core = 16384
NBLK = NODES_PC // 128    # 128-node (4-tree) blocks per core = 128
NSB = NBLK // 4           # super-blocks (16 trees, 512 nodes) = 32
NGRP = 16                 # groups (8 blocks = 32 trees = 4 GRU steps each)
BLK_PER_GRP = NBLK // NGRP
GRP_ORDER = []
for _i in range(NGRP // 2):
    GRP_ORDER += [_i, NGRP - 1 - _i]

F32 = mybir.dt.float32
F32R = mybir.dt.float32r
I32 = mybir.dt.int32
AF = mybir.ActivationFunctionType
ALU = mybir.AluOpType
AX = mybir.AxisListType

_COMPILED = {}
DEBUG = False




def _host_prep(node_ids, parent):
    """Integer-only topology preprocessing + per-core sharding layout."""
    node_ids = np.asarray(node_ids, np.int64)
    parent = np.asarray(parent, np.int64)
    ntrees = B * L
    par_loc = (parent.reshape(ntrees, N) - (np.arange(ntrees)[:, None] * N)).astype(
        np.int32
    )
    assert par_loc.min() >= 0 and par_loc.max() < N

    # AT[t, j, i] = 1 iff i is an ancestor-of-or-equal-to j within tree t.
    AT = np.zeros((ntrees, N, N), np.float32)
    ti = np.arange(ntrees)[:, None]
    jj = np.arange(N)[None, :]
    cur = np.broadcast_to(jj, (ntrees, N)).copy()
    for _ in range(12):  # depth <= 10, +margin; saturates at root 0
        AT[ti, jj, cur] = 1.0
        cur = np.take_along_axis(par_loc, cur, axis=1)
    sizes = AT.sum(axis=1).astype(np.float32)  # [ntrees, N]; s_i = |subtree(i)|

    ids_by_tree = node_ids.reshape(ntrees, N)

    idx_all, atp_all, scnt_all = [], [], []
    for c in range(NCORES):
        jcols = np.arange(TREES_PC)
        tree_of_col = (c * PB + jcols % PB) * L + jcols // PB  # [512]
        tib = tree_of_col.reshape(NBLK, 4)  # trees in block k: cols 4k..4k+4

        ids_blk = ids_by_tree[tib]  # [128, 4, 32]
        idx = ids_blk.reshape(NBLK, 128).T.astype(np.int32).copy()  # [p, k]

        at_blk = AT[tib]  # [128, 4, 32, 32]
        atk = np.zeros((NBLK, 128, 128), np.float32)
        for a in range(4):
            atk[:, a * 32 : (a + 1) * 32, a * 32 : (a + 1) * 32] = at_blk[:, a]
        atp = np.ascontiguousarray(atk.transpose(1, 0, 2))  # [p, k, q]

        sz_blk = sizes[tib]  # [128, 4, 32]
        scnt = sz_blk.reshape(NBLK * 128)

        idx_all.append(idx)
        atp_all.append(atp)
        scnt_all.append(np.ascontiguousarray(scnt))
    return idx_all, atp_all, scnt_all


def _build_kernel():
    nc = bacc.Bacc()

    emb_d = nc.declare_dram_parameter("emb", [V, E], F32R, isOutput=False)
    idx_d = nc.declare_dram_parameter("idx", [128, NBLK], I32, isOutput=False)
    atp_d = nc.declare_dram_parameter("atp", [128, NBLK, 128], F32R, isOutput=False)
    scnt_d = nc.declare_dram_parameter("scnt", [NSB * 512], F32R, isOutput=False)
    wlin_d = nc.declare_dram_parameter("w_lin", [E, C], F32R, isOutput=False)
    blin_d = nc.declare_dram_parameter("b_lin", [C], F32R, isOutput=False)
    wihT_d, whhT_d, bih_d, bhh_d = {}, {}, {}, {}
    for s in ("f", "b"):
        wihT_d[s] = nc.declare_dram_parameter(f"wihT_{s}", [C, 3 * H], F32R, isOutput=False)
        whhT_d[s] = nc.declare_dram_parameter(f"whhT_{s}", [H, 3 * H], F32R, isOutput=False)
        bih_d[s] = nc.declare_dram_parameter(f"b_ih_{s}", [3 * H], F32R, isOutput=False)
        bhh_d[s] = nc.declare_dram_parameter(f"b_hh_{s}", [3 * H], F32R, isOutput=False)
    fcw_d = nc.declare_dram_parameter("fc_w", [2 * H, OUT], F32, isOutput=False)
    fcb_d = nc.declare_dram_parameter("fc_b", [OUT], F32, isOutput=False)
    out_d = nc.declare_dram_parameter("out", [PB, OUT], F32, isOutput=True)
    if DEBUG:
        dbg_stmt_d = nc.declare_dram_parameter("dbg_stmt", [C, TREES_PC], F32, isOutput=True)
        dbg_poolf_d = nc.declare_dram_parameter("dbg_poolf", [H, PB], F32, isOutput=True)
        dbg_poolb_d = nc.declare_dram_parameter("dbg_poolb", [H, PB], F32, isOutput=True)

    with tile.TileContext(nc) as tc:
        with (
            tc.tile_pool(name="persist", bufs=1) as pp,
            tc.tile_pool(name="pa_sbuf", bufs=4) as pa,
            tc.tile_pool(name="loop", bufs=3) as pl,
            tc.tile_pool(name="psum", bufs=1, space="PSUM") as psp,
        ):
            # ---- persistent SBUF ----
            idx_sb = pp.tile([128, NBLK], I32)
            nc.sync.dma_start(idx_sb[:], idx_d[:])
            wlin_sb = pp.tile([E, C], F32R)
            nc.sync.dma_start(wlin_sb[:], wlin_d[:])
            blin_row = pp.tile([1, C], F32R)
            nc.sync.dma_start(blin_row[:], blin_d[None, :])
            scnt_sb = pp.tile([1, NSB * 512], F32R)
            nc.sync.dma_start(scnt_sb[:], scnt_d[None, :])
            stmt = pp.tile([C, TREES_PC], F32R)  # cols l-major: j = l*8 + b

            wihT_sb, whhT_sb, bih_row, bhh_row, bihn_col = {}, {}, {}, {}, {}
            for s in ("f", "b"):
                wihT_sb[s] = pp.tile([C, 3 * H], F32R, tag=f"wihT{s}", name=f"wihT{s}")
                nc.sync.dma_start(wihT_sb[s][:], wihT_d[s][:])
                whhT_sb[s] = pp.tile([H, 3 * H], F32R, tag=f"whhT{s}", name=f"whhT{s}")
                nc.sync.dma_start(whhT_sb[s][:], whhT_d[s][:])
                bih_row[s] = pp.tile([1, 3 * H], F32R, tag=f"bihr{s}", name=f"bihr{s}")
                nc.sync.dma_start(bih_row[s][:], bih_d[s][None, :])
                bhh_row[s] = pp.tile([1, 3 * H], F32R, tag=f"bhhr{s}", name=f"bhhr{s}")
                nc.sync.dma_start(bhh_row[s][:], bhh_d[s][None, :])
                bihn_col[s] = pp.tile([H, 1], F32R, tag=f"bihn{s}", name=f"bihn{s}")
                nc.sync.dma_start(bihn_col[s][:], bih_d[s][2 * H : 3 * H, None])
            fcw_sb = {}
            fcw_sb["f"] = pp.tile([H, OUT], F32, tag="fcwf", name="fcwf")
            nc.sync.dma_start(fcw_sb["f"][:], fcw_d[0:H, :])
            fcw_sb["b"] = pp.tile([H, OUT], F32, tag="fcwb", name="fcwb")
            nc.sync.dma_start(fcw_sb["b"][:], fcw_d[H : 2 * H, :])
            fcb_row = pp.tile([1, OUT], F32)
            nc.sync.dma_start(fcb_row[:], fcb_d[None, :])
            ones_row = pp.tile([1, TREES_PC], F32)
            nc.vector.memset(ones_row[:], 1.0)
            ones_r = pp.tile([1, TREES_PC], F32R)
            nc.vector.tensor_copy(out=ones_r[:], in_=ones_row[:])

            # ---- GRU PSUM banks + bias prefills ----
            rz_ps, hn_ps, gin, pooled, h_prev = {}, {}, {}, {}, {}
            for s in ("f", "b"):
                rz_ps[s] = psp.tile([128, 1024], F32, tag=f"rz{s}", name=f"rzps{s}")
                hn_ps[s] = psp.tile([128, 512], F32, tag=f"hn{s}", name=f"hnps{s}")
                # biases: r/z get b_ih+b_hh; hn bank gets b_hh_n
                for gi_, g0 in ((0, 0), (1, H)):
                    nc.tensor.matmul(
                        out=rz_ps[s][0:H, gi_ * 512 : gi_ * 512 + 512],
                        lhsT=bih_row[s][:, g0 : g0 + H],
                        rhs=ones_r[:],
                        start=True, stop=False,
                    )
                    nc.tensor.matmul(
                        out=rz_ps[s][0:H, gi_ * 512 : gi_ * 512 + 512],
                        lhsT=bhh_row[s][:, g0 : g0 + H],
                        rhs=ones_r[:],
                        start=False, stop=False, skip_group_check=True,
                    )
                nc.tensor.matmul(
                    out=hn_ps[s][0:H, :],
                    lhsT=bhh_row[s][:, 2 * H : 3 * H],
                    rhs=ones_r[:],
                    start=True, stop=False,
                )
                gin[s] = pp.tile([H, TREES_PC], F32, tag=f"gin{s}", name=f"gin{s}")
                pooled[s] = pp.tile([H, PB], F32, tag=f"pool{s}", name=f"pool{s}")
                nc.vector.memset(pooled[s][:], -1e30)
                h_prev[s] = None

            # ---- GRU step emitter (stage-interleaved across directions to
            # keep the in-order ACT/DVE queues bubble-free) ----
            def gru_pairs(steps):
                tiles = {}
                for s, t, first in steps:
                    base = t * PB
                    if not first:
                        # order n, r, z: the r->sigmoid->mul path unblocks early
                        nc.tensor.matmul(
                            out=hn_ps[s][0:H, base : base + PB],
                            lhsT=whhT_sb[s][:, 2 * H : 3 * H],
                            rhs=h_prev[s][:],
                            start=False, stop=True, skip_group_check=True,
                        )
                        for gi_, g0 in ((0, 0), (1, H)):
                            nc.tensor.matmul(
                                out=rz_ps[s][0:H, gi_ * 512 + base : gi_ * 512 + base + PB],
                                lhsT=whhT_sb[s][:, g0 : g0 + H],
                                rhs=h_prev[s][:],
                                start=False, stop=True, skip_group_check=True,
                            )
                for s, t, first in steps:
                    base = t * PB
                    rg = pl.tile([H, PB], F32, tag=f"rg{s}", name=f"rg{s}")
                    nc.scalar.activation(
                        out=rg[:], in_=rz_ps[s][0:H, base : base + PB], func=AF.Sigmoid
                    )
                    tiles[s, "rg"] = rg
                for s, t, first in steps:
                    base = t * PB
                    zg = pl.tile([H, PB], F32, tag=f"zg{s}", name=f"zg{s}")
                    nc.scalar.activation(
                        out=zg[:], in_=rz_ps[s][0:H, 512 + base : 512 + base + PB],
                        func=AF.Sigmoid,
                    )
                    tiles[s, "zg"] = zg
                for s, t, first in steps:
                    base = t * PB
                    t2 = pl.tile([H, PB], F32, tag=f"t2{s}", name=f"t2{s}")
                    nc.vector.tensor_tensor(
                        out=t2[:], in0=tiles[s, "rg"][:], in1=hn_ps[s][0:H, base : base + PB],
                        op=ALU.mult,
                    )
                    nc.vector.tensor_tensor(
                        out=t2[:], in0=t2[:], in1=gin[s][:, base : base + PB], op=ALU.add
                    )
                    tiles[s, "t2"] = t2
                for s, t, first in steps:
                    zc = pl.tile([H, PB], F32, tag=f"zc{s}", name=f"zc{s}")
                    nc.scalar.activation(
                        out=zc[:], in_=rz_ps[s][0:H, 512 + t * PB : 512 + t * PB + PB],
                        func=AF.Sigmoid, scale=-1.0,
                    )
                    tiles[s, "zc"] = zc
                for s, t, first in steps:
                    if not first:
                        zh = pl.tile([H, PB], F32, tag=f"zh{s}", name=f"zh{s}")
                        nc.vector.tensor_tensor(
                            out=zh[:], in0=tiles[s, "zg"][:],
                            in1=h_prev[s][:].bitcast(F32), op=ALU.mult,
                        )
                        tiles[s, "zh"] = zh
                for s, t, first in steps:
                    nt = pl.tile([H, PB], F32, tag=f"nt{s}", name=f"nt{s}")
                    nc.scalar.activation(out=nt[:], in_=tiles[s, "t2"][:], func=AF.Tanh)
                    tiles[s, "nt"] = nt
                for s, t, first in steps:
                    hnew = pl.tile([H, PB], F32R, tag=f"h{s}", name=f"h{s}")
                    hd = pl.tile([H, PB], F32, tag=f"hd{s}", name=f"hd{s}")
                    nc.vector.tensor_tensor(
                        out=hd[:], in0=tiles[s, "zc"][:], in1=tiles[s, "nt"][:],
                        op=ALU.mult,
                    )
                    if first:
                        nc.vector.tensor_copy(out=hnew[:], in_=hd[:])
                    else:
                        nc.vector.tensor_tensor(
                            out=hnew[:], in0=hd[:], in1=tiles[s, "zh"][:], op=ALU.add
                        )
                    nc.vector.tensor_tensor(
                        out=pooled[s][:], in0=pooled[s][:], in1=hnew[:].bitcast(F32),
                        op=ALU.max,
                    )
                    h_prev[s] = hnew
                # PE warmer: one cheap dependent matmul to keep the HAM clock
                # gate at full speed through the latency-bound GRU tail
                warm = psp.tile([128, 512], F32, tag="ps1", name="warm")
                nc.tensor.matmul(
                    out=warm[:], lhsT=wlin_sb[:], rhs=stmt[:, 0:512],
                    start=True, stop=True,
                )

            # ---- phase A groups interleaved with GRU steps ----
            done = set()
            fwd_next, bwd_next = 0, L - 1

            for g in GRP_ORDER:
                k0 = g * BLK_PER_GRP
                gbig = pa.tile([128, BLK_PER_GRP * E], F32R, tag="gbig", name="gbig")
                for bg in range(BLK_PER_GRP):
                    nc.gpsimd.indirect_dma_start(
                        out=gbig[:, bg * E : (bg + 1) * E],
                        out_offset=None,
                        in_=emb_d[:],
                        in_offset=bass.IndirectOffsetOnAxis(
                            ap=idx_sb[:, k0 + bg : k0 + bg + 1], axis=0
                        ),
                    )
                atg = pa.tile([128, BLK_PER_GRP * 128], F32R, tag="atg", name="atg")
                nc.sync.dma_start(atg[:], atp_d[:, k0 : k0 + BLK_PER_GRP, :])
                for sbl in range(BLK_PER_GRP // 4):
                    sb = g * (BLK_PER_GRP // 4) + sbl
                    ps1 = psp.tile([128, 512], F32, tag="ps1", name="ps1")
                    for a in range(4):
                        bg = 4 * sbl + a
                        nc.tensor.matmul(
                            out=ps1[:, a * 128 : (a + 1) * 128],
                            lhsT=(gbig[:, bg * E : (bg + 1) * E]),
                            rhs=(atg[:, bg * 128 : (bg + 1) * 128]),
                            start=True, stop=True,
                        )
                    m1s = pa.tile([128, 512], F32R, tag="m1s", name="m1s")
                    nc.vector.tensor_copy(out=m1s[:], in_=ps1[:])
                    ps2 = psp.tile([128, 512], F32, tag="ps2", name="ps2")
                    nc.tensor.matmul(
                        out=ps2[:], lhsT=(wlin_sb[:]), rhs=(m1s[:]),
                        start=True, stop=False,
                    )
                    nc.tensor.matmul(
                        out=ps2[:],
                        lhsT=(blin_row[:]),
                        rhs=(scnt_sb[0:1, sb * 512 : (sb + 1) * 512]),
                        start=False, stop=True, skip_group_check=True,
                    )
                    nc.vector.reduce_max(
                        out=stmt[:, sb * 16 : (sb + 1) * 16],
                        in_=ps2[:].rearrange("p (t n) -> p t n", n=32),
                        axis=AX.X,
                    )
                # gi matmuls for this group's 64 stmt cols (both directions)
                gcols = slice(32 * g, 32 * (g + 1))
                for s in ("f", "b"):
                    for gi_, g0 in ((0, 0), (1, H)):
                        nc.tensor.matmul(
                            out=rz_ps[s][0:H, gi_ * 512 + 32 * g : gi_ * 512 + 32 * (g + 1)],
                            lhsT=(wihT_sb[s][:, g0 : g0 + H]),
                            rhs=(stmt[:, gcols]),
                            start=False, stop=False, skip_group_check=True,
                        )
                    gint = psp.tile([128, 32], F32, tag="ps2", name="gint")
                    nc.tensor.matmul(
                        out=gint[0:H, :],
                        lhsT=(wihT_sb[s][:, 2 * H : 3 * H]),
                        rhs=(stmt[:, gcols]),
                        start=True, stop=True,
                    )
                    nc.vector.tensor_tensor(
                        out=gin[s][:, gcols],
                        in0=gint[0:H, :],
                        in1=bihn_col[s][:].bitcast(F32).to_broadcast([H, 32]),
                        op=ALU.add,
                    )
                done.add(g)
                # emit newly-runnable GRU steps as fwd/bwd pairs
                fs, bs = [], []
                while fwd_next <= L - 1 and (fwd_next // 4) in done:
                    fs.append(("f", fwd_next, fwd_next == 0))
                    fwd_next += 1
                while bwd_next >= 0 and (bwd_next // 4) in done:
                    bs.append(("b", bwd_next, bwd_next == L - 1))
                    bwd_next -= 1
                for i in range(max(len(fs), len(bs))):
                    batch = []
                    if i < len(fs):
                        batch.append(fs[i])
                    if i < len(bs):
                        batch.append(bs[i])
                    gru_pairs(batch)

            # ---- FC ----
            out_ps = psp.tile([PB, OUT], F32, tag="ps2", name="outps")
            nc.tensor.matmul(
                out=out_ps[:], lhsT=pooled["f"][:], rhs=fcw_sb["f"][:],
                start=True, stop=False,
            )
            nc.tensor.matmul(
                out=out_ps[:], lhsT=pooled["b"][:], rhs=fcw_sb["b"][:],
                start=False, stop=False, skip_group_check=True,
            )
            nc.tensor.matmul(
                out=out_ps[:], lhsT=ones_row[:, 0:PB], rhs=fcb_row[:],
                start=False, stop=True, skip_group_check=True,
            )
            out_sb = pp.tile([PB, OUT], F32, tag="outsb", name="outsb")
            nc.vector.tensor_copy(out=out_sb[:], in_=out_ps[:])
            nc.sync.dma_start(out_d[:], out_sb[:])
            if DEBUG:
                nc.sync.dma_start(dbg_stmt_d[:], stmt[:])
                nc.sync.dma_start(dbg_poolf_d[:], pooled["f"][:])
                nc.sync.dma_start(dbg_poolb_d[:], pooled["b"][:])

    nc.compile()
    return nc


def kernel(**inputs):
    node_ids = inputs["node_ids"]
    parent = inputs["parent"]
    emb = np.asarray(inputs["emb"], np.float32)

    idx_all, atp_all, scnt_all = _host_prep(node_ids, parent)

    if "nc" not in _COMPILED:
        _COMPILED["nc"] = _build_kernel()
    nc = _COMPILED["nc"]

    common = {
        "emb": emb,
        "w_lin": np.asarray(inputs["W_lin"], np.float32),
        "b_lin": np.asarray(inputs["b_lin"], np.float32),
        "fc_w": np.asarray(inputs["fc_w"], np.float32),
        "fc_b": np.asarray(inputs["fc_b"], np.float32),
    }
    for s in ("f", "b"):
        common[f"wihT_{s}"] = np.ascontiguousarray(
            np.asarray(inputs[f"w_ih_{s}"], np.float32).T
        )
        common[f"whhT_{s}"] = np.ascontiguousarray(
            np.asarray(inputs[f"w_hh_{s}"], np.float32).T
        )
        common[f"b_ih_{s}"] = np.asarray(inputs[f"b_ih_{s}"], np.float32)
        common[f"b_hh_{s}"] = np.asarray(inputs[f"b_hh_{s}"], np.float32)

    in_maps = []
    for c in range(NCORES):
        m = dict(common)
        m["idx"] = idx_all[c]
        m["atp"] = atp_all[c]
        m["scnt"] = scnt_all[c]
        in_maps.append(m)

    res = run_bass_kernel_spmd(nc, in_maps, list(range(NCORES)))
    _COMPILED["last_results"] = res
    out = np.concatenate([res.results[c]["out"] for c in range(NCORES)], axis=0)
    return out.astype(np.float32)


# revision 21
# speedup vs baseline: 1.0528x; 1.0528x over previous
"""ASTNN-style GNN message passing kernel for 8 Trainium2 NeuronCores.

Problem shapes (hardcoded; see module constants):
  131072 nodes = 4096 trees x 32 nodes; 4096 trees = 64 programs x 64 statements.
  Pipeline: emb gather -> linear -> bottom-up subtree sums -> per-tree max ->
  bidirectional GRU over 64 steps (batch 64) -> max pool -> FC.

Sharding: data-parallel by trees/programs. Core c owns programs [8c, 8c+8) =
512 trees = 16384 nodes. No collectives; host gathers the 8 [8,104] outputs.

Key algebraic restructure: the reference's level-by-level scatter-add equals
S = A @ (G @ W) + s * b^T per tree, where A[i,j]=1 iff j is in the subtree of
i (host-built from parent indices - integer-only preprocessing), G = gathered
embedding rows, s_i = subtree size. Per 4-tree 128-node block (no transposes):
  M1^T[e,i] = sum_j G[j,e] AT[j,i]      (PE: lhsT=G block, rhs=AT blockdiag)
  S^T[c,i]  = sum_e W[e,c] M1^T[e,i]    (PE: lhsT=W_lin as-is)
  S^T[c,i] += b[c] * s[i]               (PE: K=1 accumulate)
Per-tree max = segmented reduce_max over 32-col groups; stmt cols are l-major
(col = l*8 + b) so every GRU step's batch slice is contiguous.

GRU: gates-on-partitions layout. gi (x-side) and all biases accumulate into
PSUM banks once; each step's h-matmuls accumulate into the same banks so the
sigmoid/tanh read directly from PSUM. Groups of 64 stmt cols flow from phase
A to the GRU in order 0,7,1,6,... so the forward GRU consumes from the front
while the backward GRU consumes from the back, overlapping the gather.

Matmuls use float32r (bitcast of fp32): 1 instr instead of fp32's 2, and
1 cycle/row when the moving dim is >=256.
"""

import numpy as np

import concourse.bacc as bacc
import concourse.bass as bass
import concourse.mybir as mybir
import concourse.tile as tile
from concourse.bass_utils import run_bass_kernel_spmd

# Problem constants
B, L, N = 64, 64, 32
V, E, C, H, OUT = 100000, 128, 128, 100, 104
NCORES = 8
PB = B // NCORES          # programs per core = 8
TREES_PC = PB * L         # trees per core = 512
NODES_PC = TREES_PC * N   # nodes per core = 16384
NBLK = NODES_PC // 128    # 128-node (4-tree) blocks per core = 128
NSB = NBLK // 4           # super-blocks (16 trees, 512 nodes) = 32
NGRP = 8                  # groups (16 blocks = 64 trees = 8 GRU steps each)
BLK_PER_GRP = NBLK // NGRP
GRP_ORDER = []
for _i in range(NGRP // 2):
    GRP_ORDER += [_i, NGRP - 1 - _i]

F32 = mybir.dt.float32
F32R = mybir.dt.float32r
I32 = mybir.dt.int32
AF = mybir.ActivationFunctionType
ALU = mybir.AluOpType
AX = mybir.AxisListType

_COMPILED = {}
DEBUG = False




def _host_prep(node_ids, parent):
    """Integer-only topology preprocessing + per-core sharding layout."""
    node_ids = np.asarray(node_ids, np.int64)
    parent = np.asarray(parent, np.int64)
    ntrees = B * L
    par_loc = (parent.reshape(ntrees, N) - (np.arange(ntrees)[:, None] * N)).astype(
        np.int32
    )
    assert par_loc.min() >= 0 and par_loc.max() < N

    # AT[t, j, i] = 1 iff i is an ancestor-of-or-equal-to j within tree t.
    AT = np.zeros((ntrees, N, N), np.float32)
    ti = np.arange(ntrees)[:, None]
    jj = np.arange(N)[None, :]
    cur = np.broadcast_to(jj, (ntrees, N)).copy()
    for _ in range(12):  # depth <= 10, +margin; saturates at root 0
        AT[ti, jj, cur] = 1.0
        cur = np.take_along_axis(par_loc, cur, axis=1)
    sizes = AT.sum(axis=1).astype(np.float32)  # [ntrees, N]; s_i = |subtree(i)|

    ids_by_tree = node_ids.reshape(ntrees, N)

    idx_all, atp_all, scnt_all = [], [], []
    for c in range(NCORES):
        jcols = np.arange(TREES_PC)
        tree_of_col = (c * PB + jcols % PB) * L + jcols // PB  # [512]
        tib = tree_of_col.reshape(NBLK, 4)  # trees in block k: cols 4k..4k+4

        ids_blk = ids_by_tree[tib]  # [128, 4, 32]
        idx = ids_blk.reshape(NBLK, 128).T.astype(np.int32).copy()  # [p, k]

        at_blk = AT[tib]  # [128, 4, 32, 32]
        atk = np.zeros((NBLK, 128, 128), np.float32)
        for a in range(4):
            atk[:, a * 32 : (a + 1) * 32, a * 32 : (a + 1) * 32] = at_blk[:, a]
        atp = np.ascontiguousarray(atk.transpose(1, 0, 2))  # [p, k, q]

        sz_blk = sizes[tib]  # [128, 4, 32]
        scnt = sz_blk.reshape(NBLK * 128)

        idx_all.append(idx)
        atp_all.append(atp)
        scnt_all.append(np.ascontiguousarray(scnt))
    return idx_all, atp_all, scnt_all


def _build_kernel():
    nc = bacc.Bacc()

    emb_d = nc.declare_dram_parameter("emb", [V, E], F32R, isOutput=False)
    idx_d = nc.declare_dram_parameter("idx", [128, NBLK], I32, isOutput=False)
    atp_d = nc.declare_dram_parameter("atp", [128, NBLK, 128], F32R, isOutput=False)
    scnt_d = nc.declare_dram_parameter("scnt", [NSB * 512], F32R, isOutput=False)
    wlin_d = nc.declare_dram_parameter("w_lin", [E, C], F32R, isOutput=False)
    blin_d = nc.declare_dram_parameter("b_lin", [C], F32R, isOutput=False)
    wihT_d, whhT_d, bih_d, bhh_d = {}, {}, {}, {}
    for s in ("f", "b"):
        wihT_d[s] = nc.declare_dram_parameter(f"wihT_{s}", [C, 3 * H], F32R, isOutput=False)
        whhT_d[s] = nc.declare_dram_parameter(f"whhT_{s}", [H, 3 * H], F32R, isOutput=False)
        bih_d[s] = nc.declare_dram_parameter(f"b_ih_{s}", [3 * H], F32R, isOutput=False)
        bhh_d[s] = nc.declare_dram_parameter(f"b_hh_{s}", [3 * H], F32R, isOutput=False)
    fcw_d = nc.declare_dram_parameter("fc_w", [2 * H, OUT], F32, isOutput=False)
    fcb_d = nc.declare_dram_parameter("fc_b", [OUT], F32, isOutput=False)
    out_d = nc.declare_dram_parameter("out", [PB, OUT], F32, isOutput=True)
    if DEBUG:
        dbg_stmt_d = nc.declare_dram_parameter("dbg_stmt", [C, TREES_PC], F32, isOutput=True)
        dbg_poolf_d = nc.declare_dram_parameter("dbg_poolf", [H, PB], F32, isOutput=True)
        dbg_poolb_d = nc.declare_dram_parameter("dbg_poolb", [H, PB], F32, isOutput=True)

    with tile.TileContext(nc) as tc:
        with (
            tc.tile_pool(name="persist", bufs=1) as pp,
            tc.tile_pool(name="pa_sbuf", bufs=4) as pa,
            tc.tile_pool(name="loop", bufs=3) as pl,
            tc.tile_pool(name="psum", bufs=1, space="PSUM") as psp,
        ):
            # ---- persistent SBUF ----
            idx_sb = pp.tile([128, NBLK], I32)
            nc.sync.dma_start(idx_sb[:], idx_d[:])
            wlin_sb = pp.tile([E, C], F32R)
            nc.sync.dma_start(wlin_sb[:], wlin_d[:])
            blin_row = pp.tile([1, C], F32R)
            nc.sync.dma_start(blin_row[:], blin_d[None, :])
            scnt_sb = pp.tile([1, NSB * 512], F32R)
            nc.sync.dma_start(scnt_sb[:], scnt_d[None, :])
            stmt = pp.tile([C, TREES_PC], F32R)  # cols l-major: j = l*8 + b

            wihT_sb, whhT_sb, bih_row, bhh_row, bihn_col, bsum_row = {}, {}, {}, {}, {}, {}
            for s in ("f", "b"):
                wihT_sb[s] = pp.tile([C, 3 * H], F32R, tag=f"wihT{s}", name=f"wihT{s}")
                nc.sync.dma_start(wihT_sb[s][:], wihT_d[s][:])
                whhT_sb[s] = pp.tile([H, 3 * H], F32R, tag=f"whhT{s}", name=f"whhT{s}")
                nc.sync.dma_start(whhT_sb[s][:], whhT_d[s][:])
                bih_row[s] = pp.tile([1, 3 * H], F32R, tag=f"bihr{s}", name=f"bihr{s}")
                nc.sync.dma_start(bih_row[s][:], bih_d[s][None, :])
                bhh_row[s] = pp.tile([1, 3 * H], F32R, tag=f"bhhr{s}", name=f"bhhr{s}")
                nc.sync.dma_start(bhh_row[s][:], bhh_d[s][None, :])
                bihn_col[s] = pp.tile([H, 1], F32R, tag=f"bihn{s}", name=f"bihn{s}")
                nc.sync.dma_start(bihn_col[s][:], bih_d[s][2 * H : 3 * H, None])
                bsum_row[s] = pp.tile([1, 3 * H], F32R, tag=f"bsum{s}", name=f"bsum{s}")
                nc.vector.tensor_tensor(
                    out=bsum_row[s][:], in0=bih_row[s][:].bitcast(F32),
                    in1=bhh_row[s][:].bitcast(F32), op=ALU.add,
                )
            fcw_sb = {}
            fcw_sb["f"] = pp.tile([H, OUT], F32, tag="fcwf", name="fcwf")
            nc.sync.dma_start(fcw_sb["f"][:], fcw_d[0:H, :])
            fcw_sb["b"] = pp.tile([H, OUT], F32, tag="fcwb", name="fcwb")
            nc.sync.dma_start(fcw_sb["b"][:], fcw_d[H : 2 * H, :])
            fcb_row = pp.tile([1, OUT], F32)
            nc.sync.dma_start(fcb_row[:], fcb_d[None, :])
            ones_row = pp.tile([1, TREES_PC], F32)
            nc.vector.memset(ones_row[:], 1.0)
            ones_r = pp.tile([1, TREES_PC], F32R)
            nc.vector.tensor_copy(out=ones_r[:], in_=ones_row[:])

            # ---- GRU PSUM banks + bias prefills ----
            rz_ps, hn_ps, gin, pooled, h_prev = {}, {}, {}, {}, {}
            for s in ("f", "b"):
                rz_ps[s] = psp.tile([128, 1024], F32, tag=f"rz{s}", name=f"rzps{s}")
                hn_ps[s] = psp.tile([128, 512], F32, tag=f"hn{s}", name=f"hnps{s}")
                # biases: r/z get b_ih+b_hh; hn bank gets b_hh_n
                for gi_, g0 in ((0, 0), (1, H)):
                    nc.tensor.matmul(
                        out=rz_ps[s][0:H, gi_ * 512 : gi_ * 512 + 512],
                        lhsT=bsum_row[s][:, g0 : g0 + H],
                        rhs=ones_r[:],
                        start=True, stop=False,
                    )
                nc.tensor.matmul(
                    out=hn_ps[s][0:H, :],
                    lhsT=bhh_row[s][:, 2 * H : 3 * H],
                    rhs=ones_r[:],
                    start=True, stop=False,
                )
                gin[s] = pp.tile([H, TREES_PC], F32, tag=f"gin{s}", name=f"gin{s}")
                pooled[s] = pp.tile([H, PB], F32, tag=f"pool{s}", name=f"pool{s}")
                nc.vector.memset(pooled[s][:], -1e30)
                h_prev[s] = None

            # ---- GRU step emitter (stage-interleaved across directions to
            # keep the in-order ACT/DVE queues bubble-free) ----
            def gru_pairs(steps):
                tiles = {}
                for s, t, first in steps:
                    base = t * PB
                    if not first:
                        # order n, r, z: the r->sigmoid->mul path unblocks early
                        nc.tensor.matmul(
                            out=hn_ps[s][0:H, base : base + PB],
                            lhsT=whhT_sb[s][:, 2 * H : 3 * H],
                            rhs=h_prev[s][:],
                            start=False, stop=True, skip_group_check=True,
                        )
                        for gi_, g0 in ((0, 0), (1, H)):
                            nc.tensor.matmul(
                                out=rz_ps[s][0:H, gi_ * 512 + base : gi_ * 512 + base + PB],
                                lhsT=whhT_sb[s][:, g0 : g0 + H],
                                rhs=h_prev[s][:],
                                start=False, stop=True, skip_group_check=True,
                            )
                for s, t, first in steps:
                    base = t * PB
                    rg = pl.tile([H, PB], F32, tag=f"rg{s}", name=f"rg{s}")
                    nc.scalar.activation(
                        out=rg[:], in_=rz_ps[s][0:H, base : base + PB], func=AF.Sigmoid
                    )
                    tiles[s, "rg"] = rg
                for s, t, first in steps:
                    base = t * PB
                    zg = pl.tile([H, PB], F32, tag=f"zg{s}", name=f"zg{s}")
                    nc.scalar.activation(
                        out=zg[:], in_=rz_ps[s][0:H, 512 + base : 512 + base + PB],
                        func=AF.Sigmoid,
                    )
                    tiles[s, "zg"] = zg
                for s, t, first in steps:
                    base = t * PB
                    t2 = pl.tile([H, PB], F32, tag=f"t2{s}", name=f"t2{s}")
                    nc.vector.tensor_tensor(
                        out=t2[:], in0=tiles[s, "rg"][:], in1=hn_ps[s][0:H, base : base + PB],
                        op=ALU.mult,
                    )
                    nc.vector.tensor_tensor(
                        out=t2[:], in0=t2[:], in1=gin[s][:, base : base + PB], op=ALU.add
                    )
                    tiles[s, "t2"] = t2
                for s, t, first in steps:
                    zc = pl.tile([H, PB], F32, tag=f"zc{s}", name=f"zc{s}")
                    nc.scalar.activation(
                        out=zc[:], in_=rz_ps[s][0:H, 512 + t * PB : 512 + t * PB + PB],
                        func=AF.Sigmoid, scale=-1.0,
                    )
                    tiles[s, "zc"] = zc
                for s, t, first in steps:
                    if not first:
                        zh = pl.tile([H, PB], F32, tag=f"zh{s}", name=f"zh{s}")
                        nc.vector.tensor_tensor(
                            out=zh[:], in0=tiles[s, "zg"][:],
                            in1=h_prev[s][:].bitcast(F32), op=ALU.mult,
                        )
                        tiles[s, "zh"] = zh
                for s, t, first in steps:
                    nt = pl.tile([H, PB], F32, tag=f"nt{s}", name=f"nt{s}")
                    nc.scalar.activation(out=nt[:], in_=tiles[s, "t2"][:], func=AF.Tanh)
                    tiles[s, "nt"] = nt
                for s, t, first in steps:
                    hnew = pl.tile([H, PB], F32R, tag=f"h{s}", name=f"h{s}")
                    hd = pl.tile([H, PB], F32, tag=f"hd{s}", name=f"hd{s}")
                    nc.vector.tensor_tensor(
                        out=hd[:], in0=tiles[s, "zc"][:], in1=tiles[s, "nt"][:],
                        op=ALU.mult,
                    )
                    if first:
                        nc.vector.tensor_copy(out=hnew[:], in_=hd[:])
                    else:
                        nc.vector.tensor_tensor(
                            out=hnew[:], in0=hd[:], in1=tiles[s, "zh"][:], op=ALU.add
                        )
                    nc.vector.tensor_tensor(
                        out=pooled[s][:], in0=pooled[s][:], in1=hnew[:].bitcast(F32),
                        op=ALU.max,
                    )
                    h_prev[s] = hnew

            # ---- phase A groups interleaved with GRU steps ----
            done = set()
            fwd_next, bwd_next = 0, L - 1

            for g in GRP_ORDER:
                k0 = g * BLK_PER_GRP
                gbig = pa.tile([128, BLK_PER_GRP * E], F32R, tag="gbig", name="gbig")
                for bg in range(BLK_PER_GRP):
                    nc.gpsimd.indirect_dma_start(
                        out=gbig[:, bg * E : (bg + 1) * E],
                        out_offset=None,
                        in_=emb_d[:],
                        in_offset=bass.IndirectOffsetOnAxis(
                            ap=idx_sb[:, k0 + bg : k0 + bg + 1], axis=0
                        ),
                    )
                atg = pa.tile([128, BLK_PER_GRP * 128], F32R, tag="atg", name="atg")
                nc.sync.dma_start(atg[:], atp_d[:, k0 : k0 + BLK_PER_GRP, :])
                for sbl in range(BLK_PER_GRP // 4):
                    sb = g * (BLK_PER_GRP // 4) + sbl
                    ps1 = psp.tile([128, 512], F32, tag="ps1", name="ps1")
                    for a in range(4):
                        bg = 4 * sbl + a
                        nc.tensor.matmul(
                            out=ps1[:, a * 128 : (a + 1) * 128],
                            lhsT=(gbig[:, bg * E : (bg + 1) * E]),
                            rhs=(atg[:, bg * 128 : (bg + 1) * 128]),
                            start=True, stop=True,
                        )
                    m1s = pa.tile([128, 512], F32R, tag="m1s", name="m1s")
                    nc.vector.tensor_copy(out=m1s[:], in_=ps1[:])
                    ps2 = psp.tile([128, 512], F32, tag="ps2", name="ps2")
                    nc.tensor.matmul(
                        out=ps2[:], lhsT=(wlin_sb[:]), rhs=(m1s[:]),
                        start=True, stop=False,
                    )
                    nc.tensor.matmul(
                        out=ps2[:],
                        lhsT=blin_row[:],
                        rhs=scnt_sb[0:1, sb * 512 : (sb + 1) * 512],
                        start=False, stop=True, skip_group_check=True,
                    )
                    nc.vector.reduce_max(
                        out=stmt[:, sb * 16 : (sb + 1) * 16],
                        in_=ps2[:].rearrange("p (t n) -> p t n", n=32),
                        axis=AX.X,
                    )
                # gi matmuls for this group's 64 stmt cols (both directions)
                gcols = slice(64 * g, 64 * (g + 1))
                for s in ("f", "b"):
                    for gi_, g0 in ((0, 0), (1, H)):
                        nc.tensor.matmul(
                            out=rz_ps[s][0:H, gi_ * 512 + 64 * g : gi_ * 512 + 64 * (g + 1)],
                            lhsT=(wihT_sb[s][:, g0 : g0 + H]),
                            rhs=(stmt[:, gcols]),
                            start=False, stop=False, skip_group_check=True,
                        )
                    gint = psp.tile([128, 64], F32, tag="ps2", name="gint")
                    nc.tensor.matmul(
                        out=gint[0:H, :],
                        lhsT=(wihT_sb[s][:, 2 * H : 3 * H]),
                        rhs=(stmt[:, gcols]),
                        start=True, stop=True,
                    )
                    nc.vector.tensor_tensor(
                        out=gin[s][:, gcols],
                        in0=gint[0:H, :],
                        in1=bihn_col[s][:].bitcast(F32).to_broadcast([H, 64]),
                        op=ALU.add,
                    )
                done.add(g)
                # emit newly-runnable GRU steps as fwd/bwd pairs
                fs, bs = [], []
                while fwd_next <= L - 1 and (fwd_next // 8) in done:
                    fs.append(("f", fwd_next, fwd_next == 0))
                    fwd_next += 1
                while bwd_next >= 0 and (bwd_next // 8) in done:
                    bs.append(("b", bwd_next, bwd_next == L - 1))
                    bwd_next -= 1
                for i in range(max(len(fs), len(bs))):
                    batch = []
                    if i < len(fs):
                        batch.append(fs[i])
                    if i < len(bs):
                        batch.append(bs[i])
                    gru_pairs(batch)

            # ---- FC ----
            out_ps = psp.tile([PB, OUT], F32, tag="ps2", name="outps")
            nc.tensor.matmul(
                out=out_ps[:], lhsT=pooled["f"][:], rhs=fcw_sb["f"][:],
                start=True, stop=False,
            )
            nc.tensor.matmul(
                out=out_ps[:], lhsT=pooled["b"][:], rhs=fcw_sb["b"][:],
                start=False, stop=False, skip_group_check=True,
            )
            nc.tensor.matmul(
                out=out_ps[:], lhsT=ones_row[:, 0:PB], rhs=fcb_row[:],
                start=False, stop=True, skip_group_check=True,
            )
            out_sb = pp.tile([PB, OUT], F32, tag="outsb", name="outsb")
            nc.vector.tensor_copy(out=out_sb[:], in_=out_ps[:])
            nc.sync.dma_start(out_d[:], out_sb[:])
            if DEBUG:
                nc.sync.dma_start(dbg_stmt_d[:], stmt[:])
                nc.sync.dma_start(dbg_poolf_d[:], pooled["f"][:])
                nc.sync.dma_start(dbg_poolb_d[:], pooled["b"][:])

    nc.compile()
    return nc


def kernel(**inputs):
    node_ids = inputs["node_ids"]
    parent = inputs["parent"]
    emb = np.asarray(inputs["emb"], np.float32)

    idx_all, atp_all, scnt_all = _host_prep(node_ids, parent)

    if "nc" not in _COMPILED:
        _COMPILED["nc"] = _build_kernel()
    nc = _COMPILED["nc"]

    common = {
        "emb": emb,
        "w_lin": np.asarray(inputs["W_lin"], np.float32),
        "b_lin": np.asarray(inputs["b_lin"], np.float32),
        "fc_w": np.asarray(inputs["fc_w"], np.float32),
        "fc_b": np.asarray(inputs["fc_b"], np.float32),
    }
    for s in ("f", "b"):
        common[f"wihT_{s}"] = np.ascontiguousarray(
            np.asarray(inputs[f"w_ih_{s}"], np.float32).T
        )
        common[f"whhT_{s}"] = np.ascontiguousarray(
            np.asarray(inputs[f"w_hh_{s}"], np.float32).T
        )
        common[f"b_ih_{s}"] = np.asarray(inputs[f"b_ih_{s}"], np.float32)
        common[f"b_hh_{s}"] = np.asarray(inputs[f"b_hh_{s}"], np.float32)

    in_maps = []
    for c in range(NCORES):
        m = dict(common)
        m["idx"] = idx_all[c]
        m["atp"] = atp_all[c]
        m["scnt"] = scnt_all[c]
        in_maps.append(m)

    res = run_bass_kernel_spmd(nc, in_maps, list(range(NCORES)))
    _COMPILED["last_results"] = res
    out = np.concatenate([res.results[c]["out"] for c in range(NCORES)], axis=0)
    return out.astype(np.float32)
